# revision 2
# baseline (speedup 1.0000x reference)
"""Trainium2 Bass kernel for nn_Classifier (GNN edge classifier).

Reference, per edge e with src s=idx[0,e], dst d=idx[1,e]:
    out[e] = W2 @ relu(W1 @ [x_disease[s]; x_drug[d]] + b1) + b2

Algebraic restructure with |W2| folded into W1/b1 on the host and hidden
units permuted so positive-sign W2 entries come first (p of them):
    A'[n] = |w2| * (x_disease[n] @ W1a.T + b1)     (perm applied)
    B'[n] = |w2| * (x_drug[n]    @ W1b.T)
    out[e] = sum_{k<p} relu(A'[s]+B'[d])_k - sum_{k>=p} relu(...)_k + b2

Per-core plan (8-way data parallel over edges, 125k edges/core, padded to
131072 = 16 blocks x 8192):
  Phase B: PE matmuls build one DRAM table t_ab[20096, 512] bf16 with row
    n = [A'[n] | B'[n]] (b1 added on the A half by DVE, B half copied by
    the Scalar engine).
  Phase C per block: two 8192-idx multi-packet dma_gathers (A half via
    elem_step=512, B half at byte offset 512) -> [128, 64, 256] bf16
    tiles; DVE add (in place); Scalar-engine Relu (in place); DVE
    reduce_add over [0:p] and [p:256]; combine r1-r2+b2; DMA out
    [128, 64] f32 per block (host undoes the (p,g) interleave).

Rationale (from baseline trace): 1024-idx gathers cost ~2.9us of GpSimd
descriptor generation EACH (994ns fixed overhead dominates; 0.34ns/desc)
-> 713us serialized. 8192-idx gathers amortize the fixed cost. The
baseline's MAX,MULTIPLY stt with broadcast W2 ran at 122 G elem/s; the
fold+permute replaces it with range reduces at full DVE rate and moves
relu to the idle Scalar engine.
"""

import sys
import types
from contextlib import ExitStack

import numpy as np

import concourse.bacc as bacc
import concourse.bass as bass
import concourse.mybir as mybir
import concourse.tile as tile

F32 = mybir.dt.float32
BF16 = mybir.dt.bfloat16
I16 = mybir.dt.int16


def _cdiv(a, b):
    return (a + b - 1) // b


class Cfg:
    def __init__(self, n_nodes=20000, e_core=125000, node_chunk=2048):
        self.n_nodes = n_nodes
        self.e_core = e_core
        self.gi = 8192                      # idx per dma_gather
        self.epb = self.gi                  # 8192 edges per block
        self.nblk = _cdiv(e_core, self.epb)
        self.e_pad = self.nblk * self.epb
        self.ngrp = self.epb // 128         # 64 groups per block
        self.idx_cols = self.e_pad // 16    # wrapped idx columns per table
        self.node_chunk = node_chunk
        self.n_rows = _cdiv(n_nodes, 128) * 128


FULL = Cfg()
N_CORES = 8
E_TOTAL = 1_000_000
NQ = 4  # SWDGE queues (ucode max)
HID = 256


def build(nc, io, cfg, p_pos):
    """Emit the per-core program. p_pos = #hidden units with w2 >= 0."""
    c = cfg
    t_ab = nc.dram_tensor("t_ab", [c.n_rows, 2 * HID], BF16)

    with tile.TileContext(nc) as tc:
        with tc.tile_pool(name="const", bufs=1) as cpool:
            w_f = cpool.tile([128, 2 * HID], F32, tag="w_f")
            nc.sync.dma_start(w_f[:], io["w1abt"][:])
            wab = cpool.tile([128, 2 * HID], BF16, tag="wab")
            nc.vector.tensor_copy(wab[:], w_f[:])
            b1_sb = cpool.tile([128, HID], F32, tag="b1_sb")
            nc.sync.dma_start(b1_sb[:], io["b1bc"][:])
            b2_sb = cpool.tile([128, 1], F32, tag="b2_sb")
            nc.sync.dma_start(b2_sb[:], io["b2bc"][:])
            isrc_sb = cpool.tile([128, c.idx_cols], I16, tag="isrc")
            idst_sb = cpool.tile([128, c.idx_cols], I16, tag="idst")
            nc.sync.dma_start(isrc_sb[:], io["isrc"][:])
            nc.sync.dma_start(idst_sb[:], io["idst"][:])

            # ---- Phase B: build t_ab rows [A'[n] | B'[n]] ----
            nch = c.node_chunk
            spg = nch // 128
            with (
                tc.tile_pool(name="xb", bufs=4) as xpool,
                tc.tile_pool(name="tst", bufs=3) as spool,
                tc.tile_pool(name="ps", bufs=6, space="PSUM") as pspool,
            ):
                for ci in range(_cdiv(c.n_nodes, nch)):
                    c0 = ci * nch
                    cw = min(nch, c.n_nodes - c0)
                    xa = xpool.tile([128, nch], BF16, tag="xa")
                    xd = xpool.tile([128, nch], BF16, tag="xd")
                    # SWDGE cast-DMA f32->bf16
                    nc.gpsimd.dma_start(xa[:, :cw], io["xt_dis"][:, c0:c0 + cw])
                    nc.gpsimd.dma_start(xd[:, :cw], io["xt_drug"][:, c0:c0 + cw])
                    st = spool.tile([128, spg, 2 * HID], BF16, tag="st")
                    full_g = cw // 128
                    rem = cw % 128
                    for g in range(_cdiv(cw, 128)):
                        sw = min(128, cw - g * 128)
                        ps = pspool.tile([128, 2 * HID], F32, tag="ps")
                        nc.tensor.matmul(
                            out=ps[:sw, 0:HID],
                            lhsT=xa[:, g * 128:g * 128 + sw],
                            rhs=wab[:, 0:HID],
                            start=True, stop=True,
                        )
                        nc.tensor.matmul(
                            out=ps[:sw, HID:2 * HID],
                            lhsT=xd[:, g * 128:g * 128 + sw],
                            rhs=wab[:, HID:2 * HID],
                            start=True, stop=True,
                        )
                        nc.vector.tensor_add(
                            st[:sw, g, 0:HID], ps[:sw, 0:HID], b1_sb[:sw, :])
                        nc.scalar.copy(
                            st[:sw, g, HID:2 * HID], ps[:sw, HID:2 * HID])
                    if full_g:
                        nc.sync.dma_start(
                            t_ab[c0:c0 + full_g * 128, :].rearrange(
                                "(g p) h -> p g h", p=128),
                            st[:, :full_g, :],
                        )
                    if rem:
                        nc.sync.dma_start(
                            t_ab[c0 + full_g * 128:c0 + full_g * 128 + rem, :],
                            st[:rem, full_g, :],
                        )

            # ---- Phase C: gather + add + relu + signed range reduce ----
            wic = c.gi // 16
            with (
                tc.tile_pool(name="ga", bufs=2) as gapool,
                tc.tile_pool(name="gb", bufs=2) as gbpool,
                tc.tile_pool(name="o", bufs=3) as opool,
            ):
                for b in range(c.nblk):
                    gA = gapool.tile([128, c.ngrp, HID], BF16, tag="gA")
                    gB = gbpool.tile([128, c.ngrp, HID], BF16, tag="gB")
                    col0 = b * wic
                    nc.gpsimd.dma_gather(
                        gA[:], t_ab[:, 0:HID],
                        isrc_sb[:, col0:col0 + wic],
                        c.gi, c.gi, HID, elem_step=2 * HID,
                        single_packet=False, queue_num=(2 * b) % NQ,
                    )
                    nc.gpsimd.dma_gather(
                        gB[:], t_ab[:, HID:2 * HID],
                        idst_sb[:, col0:col0 + wic],
                        c.gi, c.gi, HID, elem_step=2 * HID,
                        single_packet=False, queue_num=(2 * b + 1) % NQ,
                    )
                    nc.vector.tensor_add(gA[:], gA[:], gB[:])
                    nc.scalar.activation(
                        gA[:], gA[:], mybir.ActivationFunctionType.Relu)
                    r1 = opool.tile([128, c.ngrp], F32, tag="r1")
                    r2 = opool.tile([128, c.ngrp], F32, tag="r2")
                    nc.vector.tensor_reduce(
                        out=r1[:], in_=gA[:, :, 0:p_pos],
                        axis=mybir.AxisListType.X, op=mybir.AluOpType.add)
                    nc.vector.tensor_reduce(
                        out=r2[:], in_=gA[:, :, p_pos:HID],
                        axis=mybir.AxisListType.X, op=mybir.AluOpType.add)
                    ob = opool.tile([128, c.ngrp], F32, tag="ob")
                    nc.vector.scalar_tensor_tensor(
                        out=ob[:], in0=r1[:], scalar=0.0, in1=r2[:],
                        op0=mybir.AluOpType.add,
                        op1=mybir.AluOpType.subtract,
                    )
                    nc.vector.tensor_scalar_add(ob[:], ob[:], b2_sb[:, 0:1])
                    nc.sync.dma_start(io["out"][b, :, :], ob[:])


# ---------------------------------------------------------------------------
# Host side
# ---------------------------------------------------------------------------

_CACHE = {}
last_result = None  # BassKernelResults of the most recent run


def _declare(nc, name, shape, dtype, is_out=False):
    return nc.declare_dram_parameter(name, list(shape), dtype, isOutput=is_out)


def _make_nc(cfg, p_pos):
    nc = bacc.Bacc("TRN2", target_bir_lowering=False, debug=False,
                   num_devices=N_CORES, num_swdge_queues=NQ,
                   detect_race_conditions=False)
    io = {
        "xt_dis": _declare(nc, "xt_dis", [128, cfg.n_nodes], F32),
        "xt_drug": _declare(nc, "xt_drug", [128, cfg.n_nodes], F32),
        "w1abt": _declare(nc, "w1abt", [128, 2 * HID], F32),
        "b1bc": _declare(nc, "b1bc", [128, HID], F32),
        "b2bc": _declare(nc, "b2bc", [128, 1], F32),
        "isrc": _declare(nc, "isrc", [128, cfg.idx_cols], I16),
        "idst": _declare(nc, "idst", [128, cfg.idx_cols], I16),
        "out": _declare(nc, "out", [cfg.nblk, 128, cfg.ngrp], F32,
                        is_out=True),
    }
    build(nc, io, cfg, p_pos)
    nc.compile()
    return nc


def _get_nc_cached(cfg, p_pos):
    key = (cfg.n_nodes, cfg.e_core, p_pos)
    if key not in _CACHE:
        _CACHE[key] = _make_nc(cfg, p_pos)
    return _CACHE[key]


def _install_ntff_hook():
    """Shim antenv.axon_hooks (absent in this image) so trace=True works."""
    import antenv
    if "antenv.axon_hooks" in sys.modules:
        return
    m = types.ModuleType("antenv.axon_hooks")
    m._hook = None
    m.set_axon_ntff_profile_hook = lambda h: setattr(m, "_hook", h)
    m.get_axon_ntff_profile_hook = lambda: m._hook
    sys.modules["antenv.axon_hooks"] = m
    antenv.axon_hooks = m
    try:
        from trn_agent_boot.trn_boot import _ntff_profile_via_ctypes
        m.set_axon_ntff_profile_hook(
            _ntff_profile_via_ctypes("/opt/axon/libaxon_pjrt.so"))
    except Exception:
        pass


def wrap_idx(idx_padded, cfg):
    """[e_pad] int16 -> [128, idx_cols] wrapped (16-row pattern x8)."""
    w = idx_padded.reshape(-1, 16).T  # logical i at [i%16, i//16]
    return np.ascontiguousarray(np.tile(w, (8, 1)))


def prep_in_maps(cfg, x_disease, x_drug, edge_label_index, W1p, b1p, b2,
                 n_cores=N_CORES):
    xt_dis = np.ascontiguousarray(x_disease.T, dtype=np.float32)
    xt_drug = np.ascontiguousarray(x_drug.T, dtype=np.float32)
    w1abt = np.ascontiguousarray(
        np.concatenate([W1p[:, :128].T, W1p[:, 128:].T], axis=1),
        dtype=np.float32)
    b1bc = np.ascontiguousarray(
        np.broadcast_to(b1p.reshape(1, HID), (128, HID)), dtype=np.float32)
    b2bc = np.full((128, 1), float(np.asarray(b2).reshape(-1)[0]), np.float32)

    e = np.asarray(edge_label_index)
    in_maps = []
    for core in range(n_cores):
        lo = core * cfg.e_core
        src = np.zeros(cfg.e_pad, np.int16)
        dst = np.zeros(cfg.e_pad, np.int16)
        src[:cfg.e_core] = e[0, lo:lo + cfg.e_core].astype(np.int16)
        dst[:cfg.e_core] = e[1, lo:lo + cfg.e_core].astype(np.int16)
        in_maps.append({
            "xt_dis": xt_dis, "xt_drug": xt_drug,
            "w1abt": w1abt, "b1bc": b1bc, "b2bc": b2bc,
            "isrc": wrap_idx(src, cfg),
            "idst": wrap_idx(dst, cfg),
        })
    return in_maps


def kernel(x_disease, x_drug, edge_label_index, W1, b1, W2, b2, _trace=False):
    global last_result
    from concourse.bass_utils import run_bass_kernel_spmd

    cfg = FULL
    if _trace:
        _install_ntff_hook()

    # Fold |w2| into W1/b1; permute hidden units so w2>=0 come first.
    w2 = np.asarray(W2, np.float32).reshape(-1)
    neg = w2 < 0
    perm = np.argsort(neg, kind="stable")
    p_pos = int((~neg).sum())
    aw = np.abs(w2)
    W1p = (np.asarray(W1, np.float32) * aw[:, None])[perm]
    b1p = (np.asarray(b1, np.float32) * aw)[perm]

    nc = _get_nc_cached(cfg, p_pos)
    in_maps = prep_in_maps(cfg, x_disease, x_drug, edge_label_index,
                           W1p, b1p, b2)
    res = run_bass_kernel_spmd(nc, in_maps, list(range(N_CORES)),
                               trace=_trace)
    last_result = res
    outs = [res.results[cr]["out"].transpose(0, 2, 1).reshape(-1)[:cfg.e_core]
            for cr in range(N_CORES)]
    return np.concatenate(outs).reshape(-1, 1).astype(np.float32)
